# revision 16
# baseline (speedup 1.0000x reference)
"""OHNM (online hard negative mining) MSE loss on 8 Trainium2 NeuronCores.

Reference computation (per map, maps = character & affinity):
    all_loss = (pred - target)^2            # N = 64*512*512 pixels
    pos_sum  = sum of all_loss * weight     # over pixels with target != 0
    num_pos  = count(target != 0)
    topk     = top-1000 of all_loss over pixels with target == 0
    k        = min(1000, 4*num_pos, num_neg)
    loss     = (pos_sum + sum(topk[:k])) / (num_pos + k)
Result = loss_character + loss_affinity  (f32 scalar).

Sharding: data-parallel over batch, 8 batches per core. Inputs are fed to the
device in bf16 (host-side cast; tolerance is 2e-2 and every sum averages the
rounding noise away), which halves HBM traffic -- the kernel is memory-bound.

Per core each map is a [128, 16384] stream processed as 4 tiles of [128, 4096]:
  ACT : n = Relu(1 - 1.2*t)   exact 0/1 negative mask (targets are 0 or >0.9),
        accum_out = per-partition negative count
  DVE : d = p - t             (tensor_tensor, bf16 2x mode)
  ACT : l = d^2
  DVE : negv = l*n            (2x; exact: n is exactly 0 or 1)
  DVE : top8 = max8(negv) -> 8 candidates per (partition, tile)
  PE  : psumA += w_blk^T @ l_blk,  psumB += w_blk^T @ negv_blk
        (32 128x128 blocks per tile, accumulated across the map's 4 tiles;
        diag(psumA) - diag(psumB) = per-column sum of w*l over positives:
        negative-pixel products are bitwise identical and cancel exactly)
Host gathers the 8 cores' partials (trace of psumA/psumB, counts, candidates)
and does the final top-k reduce over the candidate set, with an exact-numpy
fallback if the candidate set provably might miss a top-k element.
"""

import sys

sys.path.insert(0, "/opt/trn_rl_repo")

import ml_dtypes
import numpy as np

import concourse.bacc as bacc
import concourse.tile as tile
from concourse import mybir
from concourse.bass_utils import run_bass_kernel_spmd

B, C, H, W = 64, 2, 512, 512
N_CORES = 8
BPC = B // N_CORES  # batches per core
P = 128
FB = (H * W) // P  # 2048 elements per partition per batch-map
FT = 4096  # tile free size (2 batches worth per partition line)
NT = (BPC * FB) // FT  # tiles per map per core = 4
NIT = 2 * NT  # tile iterations per core (both maps) = 8
NBLK = FT // P  # 128-col blocks per tile = 32
FTOT = BPC * FB  # 16384 free elements per map per core
# chunk layout: uniform [128, 2048] chunks, 8 per map -- fine granularity
# keeps the ACT<->DVE chain shallow so compute tracks the DMA stream closely
FC = 2048
NCHUNK = 16
CHUNKS_OF_MAP = {0: list(range(8)), 1: list(range(8, 16))}
K_MAX = 1000
N_MAP = B * H * W  # pixels per map

_CACHE = {}

BF16 = ml_dtypes.bfloat16
FP8 = ml_dtypes.float8_e4m3
IDENT = np.ascontiguousarray(
    np.concatenate([np.eye(P), -np.eye(P)], axis=1).astype(BF16)
)


def _build_nc():
    f32 = mybir.dt.float32
    bf16 = mybir.dt.bfloat16
    fp8 = mybir.dt.float8e4
    AF = mybir.ActivationFunctionType
    OP = mybir.AluOpType
    nc = bacc.Bacc()
    pred = nc.declare_dram_parameter("pred", [C, P, FTOT], bf16, isOutput=False)
    cmap = nc.declare_dram_parameter("cmap", [P, FTOT], bf16, isOutput=False)
    amap = nc.declare_dram_parameter("amap", [P, FTOT], bf16, isOutput=False)
    ident = nc.declare_dram_parameter("ident", [P, 2 * P], bf16, isOutput=False)
    cw = nc.declare_dram_parameter("cw", [P, FTOT], fp8, isOutput=False)
    aw = nc.declare_dram_parameter("aw", [P, FTOT], fp8, isOutput=False)
    cand_o = nc.declare_dram_parameter("cand", [P, NCHUNK * 8], f32, isOutput=True)
    suma_o = nc.declare_dram_parameter("suma", [P, C, P], f32, isOutput=True)
    sumb_o = nc.declare_dram_parameter("sumb", [P, C, P], f32, isOutput=True)
    cnt_o = nc.declare_dram_parameter("cnts", [P, NCHUNK], f32, isOutput=True)

    # chunk work list: (map, col0, fs)
    chunks = []
    for m in range(2):
        for ti in range(FTOT // FC):
            chunks.append((m, ti * FC, FC))
    assert len(chunks) == NCHUNK

    with tile.TileContext(nc) as tc:
        with (
            tc.tile_pool(name="io", bufs=6) as io,
            tc.tile_pool(name="work", bufs=4) as work,
            tc.tile_pool(name="psum", bufs=1, space="PSUM") as psum,
            tc.tile_pool(name="dpsum", bufs=2, space="PSUM") as dpsum,
            tc.tile_pool(name="singles", bufs=1) as singles,
        ):
            candt = singles.tile([P, NCHUNK * 8], f32)
            id_t = singles.tile([P, 2 * P], bf16)
            nc.sync.dma_start(out=id_t, in_=ident[:])
            cntt = singles.tile([P, NCHUNK], f32)
            psA = [
                psum.tile([P, P], f32, tag=f"psA{m}", name=f"psA{m}")
                for m in range(2)
            ]
            psB = [
                psum.tile([P, P], f32, tag=f"psB{m}", name=f"psB{m}")
                for m in range(2)
            ]
            suma_s = [
                singles.tile([P, P], f32, tag=f"sumas{m}", name=f"sumas{m}")
                for m in range(2)
            ]
            sumb_s = [
                singles.tile([P, P], f32, tag=f"sumbs{m}", name=f"sumbs{m}")
                for m in range(2)
            ]

            maps = ((cmap, cw), (amap, aw))
            for ci, (m, col0, fs) in enumerate(chunks):
                tmap, wmap = maps[m]
                first = ci == 0 or chunks[ci - 1][0] != m
                last = ci == NCHUNK - 1 or chunks[ci + 1][0] != m
                sl = slice(col0, col0 + fs)
                p_t = io.tile([P, fs], bf16, tag="p", name="p_t")
                t_t = io.tile([P, fs], bf16, tag="t", name="t_t")
                w_t = io.tile([P, fs], fp8, tag="w", name="w_t")
                # t and p (latency-critical: they head the compute chain)
                # ride the sync HWDGE queue; w only feeds the PE stationary,
                # so it tolerates the gpsimd queue's slow semaphore path and
                # the split keeps the queues self-pacing (a single queue
                # bursts to 410+ GB/s and stalls DVE/ACT via SBUF contention)
                nc.sync.dma_start(out=t_t, in_=tmap[:, sl])
                nc.sync.dma_start(out=p_t, in_=pred[m][:, sl])
                nc.gpsimd.dma_start(out=w_t, in_=wmap[:, sl])

                # negative mask n (exact 0/1) + negative count. Alternate
                # engines to balance ACT vs DVE load: ACT uses
                # Relu(1 - 1.2*t), DVE uses is_equal(t, 0) at 4x.
                n_t = work.tile([P, fs], bf16, tag="n", name="n_t")
                if ci % 2 == 1:
                    # op1 doubles as the accumulator's reduce op
                    nc.vector.tensor_scalar(
                        out=n_t, in0=t_t, scalar1=0.0, scalar2=0.0,
                        op0=OP.is_equal, op1=OP.add,
                        accum_out=cntt[:, ci : ci + 1],
                    )
                else:
                    nc.scalar.activation(
                        out=n_t,
                        in_=t_t,
                        func=AF.Relu,
                        bias=1.0,
                        scale=-1.2,
                        accum_out=cntt[:, ci : ci + 1],
                    )

                # d = p - t on the tensor engine: psum = I^T @ p + (-I)^T @ t
                # in 512-col groups (frees DVE, whose MAX8+negv load is the
                # critical path). ACT squares straight out of PSUM.
                l_t = work.tile([P, fs], bf16, tag="l", name="l_t")
                for h in range(fs // 1024):
                    d_ps = dpsum.tile([P, 1024], f32, tag="dps", name="d_ps")
                    for cg in range(2):
                        csl = slice(h * 1024 + cg * 512, h * 1024 + cg * 512 + 512)
                        osl = slice(cg * 512, cg * 512 + 512)
                        nc.tensor.matmul(
                            d_ps[:, osl], id_t[:, :P], p_t[:, csl],
                            start=True, stop=False,
                        )
                        nc.tensor.matmul(
                            d_ps[:, osl], id_t[:, P:], t_t[:, csl],
                            start=False, stop=True,
                        )
                    # l = d^2 (PSUM -> SBUF)
                    nc.scalar.square(l_t[:, h * 1024 : (h + 1) * 1024], d_ps)

                # negv = l*n: exact 0 at positives, exact copy of l at
                # negatives (n is exactly 1.0 there)
                negv = work.tile([P, fs], bf16, tag="negv", name="negv")
                nc.vector.tensor_mul(negv, l_t, n_t)

                # top-8 negative losses per (partition, chunk)
                nc.vector.max(out=candt[:, ci * 8 : (ci + 1) * 8], in_=negv)

                # PE: accumulate w^T @ l and w^T @ negv in 128x128 blocks;
                # only the diagonals are used (per-column dot products)
                nblk = fs // P
                for bk in range(nblk):
                    bsl = slice(bk * P, (bk + 1) * P)
                    nc.tensor.matmul(
                        psA[m],
                        w_t[:, bsl],
                        l_t[:, bsl],
                        start=first and bk == 0,
                        stop=last and bk == nblk - 1,
                    )
                for bk in range(nblk):
                    bsl = slice(bk * P, (bk + 1) * P)
                    nc.tensor.matmul(
                        psB[m],
                        w_t[:, bsl],
                        negv[:, bsl],
                        start=first and bk == 0,
                        stop=last and bk == nblk - 1,
                    )
                if last:
                    # drain this map's PSUM accumulators right away so the
                    # final output DMAs overlap the other map's stream
                    nc.scalar.copy(suma_s[m], psA[m])
                    nc.scalar.copy(sumb_s[m], psB[m])
                    nc.sync.dma_start(out=suma_o[:, m], in_=suma_s[m])
                    nc.sync.dma_start(out=sumb_o[:, m], in_=sumb_s[m])

            nc.sync.dma_start(out=cand_o[:], in_=candt)
            nc.sync.dma_start(out=cnt_o[:], in_=cntt)
    nc.compile()
    return nc


def _get_nc():
    if "nc" not in _CACHE:
        _CACHE["nc"] = _build_nc()
    return _CACHE["nc"]


def _ohnm_np(pred, target, weight):
    """Exact numpy fallback, mirrors the reference."""
    all_loss = (pred - target) ** 2
    pos_mask = target != 0
    num_pos = int(pos_mask.sum())
    num_neg = pred.size - num_pos
    pos_sum = float((all_loss * weight)[pos_mask].astype(np.float64).sum())
    neg_loss = np.where(pos_mask, -np.inf, all_loss)
    k = min(K_MAX, 4 * num_pos, num_neg)
    topk = np.sort(neg_loss.ravel())[-K_MAX:][::-1]
    neg_sum = float(topk[:k].astype(np.float64).sum())
    return np.float32((pos_sum + neg_sum) / np.float64(num_pos + k))


def _to_core_layout(arr_core):
    """[BPC, H, W] f32 -> [P, FTOT] bf16 with each partition holding BPC
    contiguous per-batch segments."""
    a = arr_core.reshape(BPC, P, FB).transpose(1, 0, 2).reshape(P, FTOT)
    return np.ascontiguousarray(a.astype(BF16))


def _to_core_layout_fp8(arr_core):
    a = arr_core.reshape(BPC, P, FB).transpose(1, 0, 2).reshape(P, FTOT)
    return np.ascontiguousarray(a.astype(FP8))


def _combine_map(results, m):
    """Host-side final reduce for one map from the 8 cores' partials."""
    pos_sum = 0.0
    num_neg = 0.0
    cands = []
    cc = CHUNKS_OF_MAP[m]
    for r in results:
        da = np.diagonal(np.asarray(r["suma"])[:, m]).astype(np.float64)
        db = np.diagonal(np.asarray(r["sumb"])[:, m]).astype(np.float64)
        pos_sum += float(da.sum() - db.sum())
        num_neg += float(np.asarray(r["cnts"])[:, cc].astype(np.float64).sum())
        cands.append(
            np.asarray(r["cand"])[:, cc[0] * 8 : (cc[-1] + 1) * 8]
            .astype(np.float32)
            .reshape(P, len(cc), 8)
        )
    cand = np.stack(cands)  # [cores, P, nchunks, 8] descending within chunks
    num_neg = int(round(num_neg))
    num_pos = N_MAP - num_neg
    k = min(K_MAX, 4 * num_pos, num_neg)
    flat = np.sort(cand.ravel())[::-1]
    neg_sum = float(flat[:k].astype(np.float64).sum()) if k > 0 else 0.0
    ok = True
    if k > 0:
        tau = flat[k - 1]
        # A chunk can only hide a missed top-k element if its own 8th-largest
        # (the smallest we kept) is strictly above the k-th candidate.
        chunk_min = cand[..., 7]
        ok = not bool((chunk_min > tau).any())
    loss = np.float32((pos_sum + neg_sum) / np.float64(num_pos + k))
    return loss, ok


def make_in_maps(output, character_map, affinity_map, character_weight, affinity_weight):
    in_maps = []
    for i in range(N_CORES):
        sl = slice(i * BPC, (i + 1) * BPC)
        pred_core = np.stack(
            [
                _to_core_layout(output[sl, 0]),
                _to_core_layout(output[sl, 1]),
            ]
        )
        in_maps.append(
            {
                "ident": IDENT,
                "pred": pred_core,
                "cmap": _to_core_layout(character_map[sl]),
                "amap": _to_core_layout(affinity_map[sl]),
                "cw": _to_core_layout_fp8(character_weight[sl]),
                "aw": _to_core_layout_fp8(affinity_weight[sl]),
            }
        )
    return in_maps


def kernel(output, character_map, affinity_map, character_weight, affinity_weight):
    output = np.asarray(output, dtype=np.float32)
    character_map = np.asarray(character_map, dtype=np.float32)
    affinity_map = np.asarray(affinity_map, dtype=np.float32)
    character_weight = np.asarray(character_weight, dtype=np.float32)
    affinity_weight = np.asarray(affinity_weight, dtype=np.float32)

    nc = _get_nc()
    in_maps = make_in_maps(
        output, character_map, affinity_map, character_weight, affinity_weight
    )
    results = run_bass_kernel_spmd(nc, in_maps, list(range(N_CORES))).results

    loss_c, ok_c = _combine_map(results, 0)
    loss_a, ok_a = _combine_map(results, 1)
    if not ok_c:
        flat = output.transpose(0, 2, 3, 1).reshape(-1, C)
        loss_c = _ohnm_np(
            flat[:, 0], character_map.reshape(-1), character_weight.reshape(-1)
        )
    if not ok_a:
        flat = output.transpose(0, 2, 3, 1).reshape(-1, C)
        loss_a = _ohnm_np(
            flat[:, 1], affinity_map.reshape(-1), affinity_weight.reshape(-1)
        )
    return np.array(np.float32(loss_c) + np.float32(loss_a), dtype=np.float32)


# revision 17
# speedup vs baseline: 1.0762x; 1.0762x over previous
"""OHNM (online hard negative mining) MSE loss on 8 Trainium2 NeuronCores.

Reference computation (per map, maps = character & affinity):
    all_loss = (pred - target)^2            # N = 64*512*512 pixels
    pos_sum  = sum of all_loss * weight     # over pixels with target != 0
    num_pos  = count(target != 0)
    topk     = top-1000 of all_loss over pixels with target == 0
    k        = min(1000, 4*num_pos, num_neg)
    loss     = (pos_sum + sum(topk[:k])) / (num_pos + k)
Result = loss_character + loss_affinity  (f32 scalar).

Sharding: data-parallel over batch, 8 batches per core. Inputs are fed to the
device in bf16 (host-side cast; tolerance is 2e-2 and every sum averages the
rounding noise away), which halves HBM traffic -- the kernel is memory-bound.

Per core each map is a [128, 16384] stream processed as 4 tiles of [128, 4096]:
  ACT : n = Relu(1 - 1.2*t)   exact 0/1 negative mask (targets are 0 or >0.9),
        accum_out = per-partition negative count
  DVE : d = p - t             (tensor_tensor, bf16 2x mode)
  ACT : l = d^2
  DVE : negv = l*n            (2x; exact: n is exactly 0 or 1)
  DVE : top8 = max8(negv) -> 8 candidates per (partition, tile)
  PE  : psumA += w_blk^T @ l_blk,  psumB += w_blk^T @ negv_blk
        (32 128x128 blocks per tile, accumulated across the map's 4 tiles;
        diag(psumA) - diag(psumB) = per-column sum of w*l over positives:
        negative-pixel products are bitwise identical and cancel exactly)
Host gathers the 8 cores' partials (trace of psumA/psumB, counts, candidates)
and does the final top-k reduce over the candidate set, with an exact-numpy
fallback if the candidate set provably might miss a top-k element.
"""

import sys

sys.path.insert(0, "/opt/trn_rl_repo")

import ml_dtypes
import numpy as np

import concourse.bacc as bacc
import concourse.tile as tile
from concourse import mybir
from concourse.bass_utils import run_bass_kernel_spmd

B, C, H, W = 64, 2, 512, 512
N_CORES = 8
BPC = B // N_CORES  # batches per core
P = 128
FB = (H * W) // P  # 2048 elements per partition per batch-map
FT = 4096  # tile free size (2 batches worth per partition line)
NT = (BPC * FB) // FT  # tiles per map per core = 4
NIT = 2 * NT  # tile iterations per core (both maps) = 8
NBLK = FT // P  # 128-col blocks per tile = 32
FTOT = BPC * FB  # 16384 free elements per map per core
# chunk layout: [128, 2048] chunks, with the first and last chunks split in
# half -- a small first chunk starts the compute chain sooner after its DMA,
# and a small last chunk keeps the end-of-kernel dependency chain short
FC = 2048
NCHUNK = 18
CHUNKS_OF_MAP = {0: list(range(9)), 1: list(range(9, 18))}
K_MAX = 1000
N_MAP = B * H * W  # pixels per map

_CACHE = {}

BF16 = ml_dtypes.bfloat16
FP8 = ml_dtypes.float8_e4m3


def _build_nc():
    f32 = mybir.dt.float32
    bf16 = mybir.dt.bfloat16
    fp8 = mybir.dt.float8e4
    AF = mybir.ActivationFunctionType
    nc = bacc.Bacc()
    pred = nc.declare_dram_parameter("pred", [C, P, FTOT], bf16, isOutput=False)
    cmap = nc.declare_dram_parameter("cmap", [P, FTOT], bf16, isOutput=False)
    amap = nc.declare_dram_parameter("amap", [P, FTOT], bf16, isOutput=False)
    cw = nc.declare_dram_parameter("cw", [P, FTOT], fp8, isOutput=False)
    aw = nc.declare_dram_parameter("aw", [P, FTOT], fp8, isOutput=False)
    cand_o = nc.declare_dram_parameter("cand", [P, NCHUNK * 8], f32, isOutput=True)
    suma_o = nc.declare_dram_parameter("suma", [P, C, P], f32, isOutput=True)
    sumb_o = nc.declare_dram_parameter("sumb", [P, C, P], f32, isOutput=True)
    cnt_o = nc.declare_dram_parameter("cnts", [P, NCHUNK], f32, isOutput=True)

    # chunk work list: (map, col0, fs)
    chunks = []
    for m in range(2):
        for ti in range(FTOT // FC):
            if m == 0 and ti == 0:
                chunks.append((m, 0, FC // 2))
                chunks.append((m, FC // 2, FC // 2))
            elif m == 1 and ti == FTOT // FC - 1:
                chunks.append((m, ti * FC, FC // 2))
                chunks.append((m, ti * FC + FC // 2, FC // 2))
            else:
                chunks.append((m, ti * FC, FC))
    assert len(chunks) == NCHUNK

    with tile.TileContext(nc) as tc:
        with (
            tc.tile_pool(name="io", bufs=8) as io,
            tc.tile_pool(name="work", bufs=6) as work,
            tc.tile_pool(name="psum", bufs=1, space="PSUM") as psum,
            tc.tile_pool(name="singles", bufs=1) as singles,
        ):
            candt = singles.tile([P, NCHUNK * 8], f32)
            cntt = singles.tile([P, NCHUNK], f32)
            psA = [
                psum.tile([P, P], f32, tag=f"psA{m}", name=f"psA{m}")
                for m in range(2)
            ]
            psB = [
                psum.tile([P, P], f32, tag=f"psB{m}", name=f"psB{m}")
                for m in range(2)
            ]
            suma_s = [
                singles.tile([P, P], f32, tag=f"sumas{m}", name=f"sumas{m}")
                for m in range(2)
            ]
            sumb_s = [
                singles.tile([P, P], f32, tag=f"sumbs{m}", name=f"sumbs{m}")
                for m in range(2)
            ]

            maps = ((cmap, cw), (amap, aw))
            for ci, (m, col0, fs) in enumerate(chunks):
                tmap, wmap = maps[m]
                first = ci == 0 or chunks[ci - 1][0] != m
                last = ci == NCHUNK - 1 or chunks[ci + 1][0] != m
                sl = slice(col0, col0 + fs)
                p_t = io.tile([P, fs], bf16, tag="p", name="p_t")
                t_t = io.tile([P, fs], bf16, tag="t", name="t_t")
                w_t = io.tile([P, fs], fp8, tag="w", name="w_t")
                # t and p (latency-critical: they head the compute chain)
                # ride the sync HWDGE queue; w only feeds the PE stationary,
                # so it tolerates the gpsimd queue's slow semaphore path and
                # the split keeps the queues self-pacing (a single queue
                # bursts to 410+ GB/s and stalls DVE/ACT via SBUF contention)
                nc.sync.dma_start(out=t_t, in_=tmap[:, sl])
                nc.sync.dma_start(out=p_t, in_=pred[m][:, sl])
                nc.gpsimd.dma_start(out=w_t, in_=wmap[:, sl])

                # n = Relu(1 - 1.2*t): exactly 1 at negatives (t == 0),
                # exactly 0 at positives (t > 0.89 even after bf16 rounding);
                # accum = negative count
                n_t = work.tile([P, fs], bf16, tag="n", name="n_t")
                nc.scalar.activation(
                    out=n_t,
                    in_=t_t,
                    func=AF.Relu,
                    bias=1.0,
                    scale=-1.2,
                    accum_out=cntt[:, ci : ci + 1],
                )

                # d = p - t (bf16 tensor_tensor, 2x mode). NOTE: do not
                # offload to gpsimd -- its SBUF traffic stalls concurrent
                # DVE ops to ~1/4 speed (measured), a net loss.
                d_t = work.tile([P, fs], bf16, tag="d", name="d_t")
                nc.vector.tensor_sub(d_t, p_t, t_t)

                # l = d^2 on ACT
                l_t = work.tile([P, fs], bf16, tag="l", name="l_t")
                nc.scalar.square(l_t, d_t)

                # negv = l*n: exact 0 at positives, exact copy of l at
                # negatives (n is exactly 1.0 there)
                negv = work.tile([P, fs], bf16, tag="negv", name="negv")
                nc.vector.tensor_mul(negv, l_t, n_t)

                # top-8 negative losses per (partition, chunk)
                nc.vector.max(out=candt[:, ci * 8 : (ci + 1) * 8], in_=negv)

                # PE: accumulate w^T @ l and w^T @ negv in 128x128 blocks;
                # only the diagonals are used (per-column dot products)
                nblk = fs // P
                for bk in range(nblk):
                    bsl = slice(bk * P, (bk + 1) * P)
                    nc.tensor.matmul(
                        psA[m],
                        w_t[:, bsl],
                        l_t[:, bsl],
                        start=first and bk == 0,
                        stop=last and bk == nblk - 1,
                    )
                for bk in range(nblk):
                    bsl = slice(bk * P, (bk + 1) * P)
                    nc.tensor.matmul(
                        psB[m],
                        w_t[:, bsl],
                        negv[:, bsl],
                        start=first and bk == 0,
                        stop=last and bk == nblk - 1,
                    )
                if last:
                    # drain this map's PSUM accumulators right away so the
                    # final output DMAs overlap the other map's stream
                    nc.scalar.copy(suma_s[m], psA[m])
                    nc.scalar.copy(sumb_s[m], psB[m])
                    nc.sync.dma_start(out=suma_o[:, m], in_=suma_s[m])
                    nc.sync.dma_start(out=sumb_o[:, m], in_=sumb_s[m])

            nc.sync.dma_start(out=cand_o[:], in_=candt)
            nc.sync.dma_start(out=cnt_o[:], in_=cntt)
    nc.compile()
    return nc


def _get_nc():
    if "nc" not in _CACHE:
        _CACHE["nc"] = _build_nc()
    return _CACHE["nc"]


def _ohnm_np(pred, target, weight):
    """Exact numpy fallback, mirrors the reference."""
    all_loss = (pred - target) ** 2
    pos_mask = target != 0
    num_pos = int(pos_mask.sum())
    num_neg = pred.size - num_pos
    pos_sum = float((all_loss * weight)[pos_mask].astype(np.float64).sum())
    neg_loss = np.where(pos_mask, -np.inf, all_loss)
    k = min(K_MAX, 4 * num_pos, num_neg)
    topk = np.sort(neg_loss.ravel())[-K_MAX:][::-1]
    neg_sum = float(topk[:k].astype(np.float64).sum())
    return np.float32((pos_sum + neg_sum) / np.float64(num_pos + k))


def _to_core_layout(arr_core):
    """[BPC, H, W] f32 -> [P, FTOT] bf16 with each partition holding BPC
    contiguous per-batch segments."""
    a = arr_core.reshape(BPC, P, FB).transpose(1, 0, 2).reshape(P, FTOT)
    return np.ascontiguousarray(a.astype(BF16))


def _to_core_layout_fp8(arr_core):
    a = arr_core.reshape(BPC, P, FB).transpose(1, 0, 2).reshape(P, FTOT)
    return np.ascontiguousarray(a.astype(FP8))


def _combine_map(results, m):
    """Host-side final reduce for one map from the 8 cores' partials."""
    pos_sum = 0.0
    num_neg = 0.0
    cands = []
    cc = CHUNKS_OF_MAP[m]
    for r in results:
        da = np.diagonal(np.asarray(r["suma"])[:, m]).astype(np.float64)
        db = np.diagonal(np.asarray(r["sumb"])[:, m]).astype(np.float64)
        pos_sum += float(da.sum() - db.sum())
        num_neg += float(np.asarray(r["cnts"])[:, cc].astype(np.float64).sum())
        cands.append(
            np.asarray(r["cand"])[:, cc[0] * 8 : (cc[-1] + 1) * 8]
            .astype(np.float32)
            .reshape(P, len(cc), 8)
        )
    cand = np.stack(cands)  # [cores, P, nchunks, 8] descending within chunks
    num_neg = int(round(num_neg))
    num_pos = N_MAP - num_neg
    k = min(K_MAX, 4 * num_pos, num_neg)
    flat = np.sort(cand.ravel())[::-1]
    neg_sum = float(flat[:k].astype(np.float64).sum()) if k > 0 else 0.0
    ok = True
    if k > 0:
        tau = flat[k - 1]
        # A chunk can only hide a missed top-k element if its own 8th-largest
        # (the smallest we kept) is strictly above the k-th candidate.
        chunk_min = cand[..., 7]
        ok = not bool((chunk_min > tau).any())
    loss = np.float32((pos_sum + neg_sum) / np.float64(num_pos + k))
    return loss, ok


def make_in_maps(output, character_map, affinity_map, character_weight, affinity_weight):
    in_maps = []
    for i in range(N_CORES):
        sl = slice(i * BPC, (i + 1) * BPC)
        pred_core = np.stack(
            [
                _to_core_layout(output[sl, 0]),
                _to_core_layout(output[sl, 1]),
            ]
        )
        in_maps.append(
            {
                "pred": pred_core,
                "cmap": _to_core_layout(character_map[sl]),
                "amap": _to_core_layout(affinity_map[sl]),
                "cw": _to_core_layout_fp8(character_weight[sl]),
                "aw": _to_core_layout_fp8(affinity_weight[sl]),
            }
        )
    return in_maps


def kernel(output, character_map, affinity_map, character_weight, affinity_weight):
    output = np.asarray(output, dtype=np.float32)
    character_map = np.asarray(character_map, dtype=np.float32)
    affinity_map = np.asarray(affinity_map, dtype=np.float32)
    character_weight = np.asarray(character_weight, dtype=np.float32)
    affinity_weight = np.asarray(affinity_weight, dtype=np.float32)

    nc = _get_nc()
    in_maps = make_in_maps(
        output, character_map, affinity_map, character_weight, affinity_weight
    )
    results = run_bass_kernel_spmd(nc, in_maps, list(range(N_CORES))).results

    loss_c, ok_c = _combine_map(results, 0)
    loss_a, ok_a = _combine_map(results, 1)
    if not ok_c:
        flat = output.transpose(0, 2, 3, 1).reshape(-1, C)
        loss_c = _ohnm_np(
            flat[:, 0], character_map.reshape(-1), character_weight.reshape(-1)
        )
    if not ok_a:
        flat = output.transpose(0, 2, 3, 1).reshape(-1, C)
        loss_a = _ohnm_np(
            flat[:, 1], affinity_map.reshape(-1), affinity_weight.reshape(-1)
        )
    return np.array(np.float32(loss_c) + np.float32(loss_a), dtype=np.float32)
